# revision 1
# baseline (speedup 1.0000x reference)
"""Trainium2 Bass kernel for nn_BERTSyntaxRel (biaffine syntax-relation head).

Computation (per batch b, token t):
    appended = concat([root, x[b]])                      # (S+1, D)
    gathered = appended[head_id[b, t]]                   # (D,)
    head = relu(gathered @ Wh + bh)                      # (H,)
    tail = relu(x[b, t] @ Wt + bt)                       # (H,)
    out[b, t, r] = sum_{h,k} head[h] * K[h, r, k] * tail[k]

Sharding: data-parallel over batch, 4 batches per core on 8 cores.
Weights replicated.  No collectives needed.

Key restructure: the gather commutes with the row-wise head FF, so we
compute head_all = relu([root; x[b]] @ Wh + bh) for ALL positions first
(same FLOPs), write the (1025, H) per-batch table to DRAM, and gather
H=128-float rows instead of D=768-float x rows.
"""

import numpy as np

B, S, D, H, R = 32, 1024, 768, 128, 48
NCORES = 8
BPC = B // NCORES            # batches per core (4)
TOK = BPC * S                # tokens per core (4096)
P = 128                      # partition dim / token tile
NTILES = TOK // P            # 32 token tiles per core
TBL = S + 1                  # rows per batch gather table (1025)
DC = D // P                  # 6 contraction chunks of 128
RKCH = 12                    # biaffine free-dim chunks of 512 (R*H = 6144)
RPC = 4                      # r values per 512-chunk
import os as _os

NDVE = int(_os.environ.get("K_NDVE", "4"))  # chunks via DVE mul+reduce;
                             # the rest go ACT-copy -> gpsimd-mul -> DVE-reduce
DEPEDGE = _os.environ.get("K_DEPEDGE", "1") == "1"  # explicit gather->table-write deps
ILV = _os.environ.get("K_ILV", "1") == "1"  # interleave Phase A/B emission per batch
STT = _os.environ.get("K_STT", "0") == "1"  # fused scalar_tensor_tensor consume on DVE
PAIR = _os.environ.get("K_PAIR", "0") == "1"  # 2-bank (P,1024) consume chunks
MERGEAF = _os.environ.get("K_MERGEAF", "1") == "1"  # share psA/psF slots, psM=5
HTA = _os.environ.get("K_HTA", "0") == "1"  # gather-transpose PSUM from the A pool


def build_program(with_bias=True):
    """Build the Bass program (shared by all 8 cores, SPMD)."""
    from contextlib import ExitStack

    import concourse.bass as bass
    import concourse.tile as tile
    from concourse import bacc, mybir
    from concourse.masks import make_identity

    f32 = mybir.dt.float32
    i32 = mybir.dt.int32
    ts = bass.ts

    nc = bacc.Bacc(
        "TRN2",
        target_bir_lowering=False,
        debug=False,
        num_devices=NCORES,
    )

    x_ap = nc.dram_tensor("x", [TOK, D], f32, kind="ExternalInput").ap()
    gidx_ap = nc.dram_tensor("gidx", [TOK, 1], i32, kind="ExternalInput").ap()
    wh_ap = nc.dram_tensor("Wh", [D, H], f32, kind="ExternalInput").ap()
    wt_ap = nc.dram_tensor("Wt", [D, H], f32, kind="ExternalInput").ap()
    bh_ap = nc.dram_tensor("bh", [1, H], f32, kind="ExternalInput").ap()
    bt_ap = nc.dram_tensor("bt", [1, H], f32, kind="ExternalInput").ap()
    rooth_ap = nc.dram_tensor("rooth", [1, H], f32, kind="ExternalInput").ap()
    kern_ap = nc.dram_tensor("kern", [H, R * H], f32, kind="ExternalInput").ap()
    out_ap = nc.dram_tensor("out", [TOK, R], f32, kind="ExternalOutput").ap()

    with tile.TileContext(nc) as tc, ExitStack() as ctx:
        # ---- constants / weights, resident for the whole kernel ----
        const = ctx.enter_context(tc.tile_pool(name="const", bufs=1))
        ident = const.tile([P, P], f32)
        make_identity(nc, ident[:])
        ones1 = const.tile([1, P], f32)
        nc.gpsimd.memset(ones1[:], 1.0)
        # combined FF weights: per d-chunk c, wht[:, c*256 : c*256+128] = Wh chunk,
        # wht[:, c*256+128 : (c+1)*256] = Wt chunk -> one N=256 matmul per chunk
        wht = const.tile([P, 2 * D], f32)
        wh3 = wh_ap.rearrange("(c p) h -> c p h", p=P)
        wt3 = wt_ap.rearrange("(c p) h -> c p h", p=P)
        for c in range(DC):
            nc.sync.dma_start(out=wht[:, ts(2 * c, P)], in_=wh3[c])
            nc.sync.dma_start(out=wht[:, ts(2 * c + 1, P)], in_=wt3[c])
        bb_sb = const.tile([1, 2 * H], f32)
        rt_sb = const.tile([1, H], f32)
        nc.sync.dma_start(out=bb_sb[:, :H], in_=bh_ap[:])
        nc.sync.dma_start(out=bb_sb[:, H:], in_=bt_ap[:])
        nc.sync.dma_start(out=rt_sb[:], in_=rooth_ap[:])
        ksb = const.tile([H, R * H], f32)  # 24KB/partition
        nc.sync.dma_start(out=ksb[:], in_=kern_ap[:])

        # tail (tok-major) for the whole core, kept in SBUF: 16KB/partition
        tailT_all = const.tile([P, TOK], f32)

        # per-batch gather tables in DRAM: row b*TBL is the root head state
        dram = ctx.enter_context(tc.tile_pool(name="dram", bufs=1, space="DRAM"))
        head_all = dram.tile([BPC * TBL, H], f32)
        # head_all writers per batch (Tile does not track DRAM deps; the
        # Phase-B gathers get explicit dep edges on these)
        tbl_writes = [[] for _ in range(BPC)]
        for b in range(BPC):
            w = nc.sync.dma_start(
                out=head_all[b * TBL : b * TBL + 1, :], in_=rt_sb[:1, :]
            )
            tbl_writes[b].append(w.ins)

        # ---- Phase A: transposes + FFs; fills head_all (DRAM) and tailT_all ----
        def emit_A(i, xa_pool, xt_pool, ha_pool, psA, psF):
                b = i // (S // P)
                xt = xa_pool.tile([P, D], f32)
                nc.sync.dma_start(out=xt[:], in_=x_ap[ts(i, P), :])
                xT = xt_pool.tile([P, D], f32)
                if ILV:
                    # 1-bank PSUM tiles so Phase A+B pools fit in 8 banks
                    for half in range(2):
                        xT_ps = psA.tile([P, D // 2], f32, tag="psa")
                        for c3 in range(DC // 2):
                            c = half * (DC // 2) + c3
                            nc.tensor.transpose(
                                out=xT_ps[:, ts(c3, P)], in_=xt[:, ts(c, P)],
                                identity=ident[:],
                            )
                        nc.scalar.copy(out=xT[:, ts(half, D // 2)], in_=xT_ps[:])
                else:
                    xT_ps = psA.tile([P, D], f32)
                    for c in range(DC):
                        nc.tensor.transpose(
                            out=xT_ps[:, ts(c, P)], in_=xt[:, ts(c, P)],
                            identity=ident[:],
                        )
                    nc.scalar.copy(out=xT[:], in_=xT_ps[:])

                ff_ps = psF.tile([P, 2 * H], f32, tag="psa" if (ILV and psF is psA) else "ff_ps")
                for c in range(DC):
                    nc.tensor.matmul(
                        out=ff_ps[:], lhsT=xT[:, ts(c, P)], rhs=wht[:, ts(c, 2 * P)],
                        start=(c == 0), stop=(c == DC - 1 and not with_bias),
                    )
                if with_bias:
                    nc.tensor.matmul(
                        out=ff_ps[:], lhsT=ones1[:1, :], rhs=bb_sb[:1, :],
                        start=False, stop=True,
                    )
                hA = ha_pool.tile([P, H], f32)
                nc.scalar.activation(
                    out=hA[:], in_=ff_ps[:, :H], func=mybir.ActivationFunctionType.Relu
                )
                nc.scalar.activation(
                    out=tailT_all[:, ts(i, P)], in_=ff_ps[:, H:],
                    func=mybir.ActivationFunctionType.Relu,
                )
                row0 = b * TBL + 1 + (i % (S // P)) * P
                w = nc.sync.dma_start(out=head_all[row0 : row0 + P, :], in_=hA[:])
                tbl_writes[b].append(w.ins)

        # ---- Phase B: gather + biaffine ----
        def emit_B(i, gx_pool, gb_pool, hb_pool, prod_pool, ob_pool, psT, psM):
                b = i // (S // P)
                gix = gx_pool.tile([P, 1], i32)
                nc.sync.dma_start(out=gix[:], in_=gidx_ap[ts(i, P), :])
                g_sb = gb_pool.tile([P, H], f32)
                g = nc.gpsimd.indirect_dma_start(
                    out=g_sb[:],
                    out_offset=None,
                    in_=head_all[:],
                    in_offset=bass.IndirectOffsetOnAxis(ap=gix[:, :1], axis=0),
                )
                if DEPEDGE or ILV:  # mandatory when there is no phase barrier
                    for w_ins in tbl_writes[b]:
                        tile.add_dep_helper(
                            g.ins, w_ins, sync=True, reason="head_all RAW"
                        )
                hT_tag = "psa" if (ILV and psT is not psM) else (
                    "tmp_ps" if psT is psM else "hT_ps"
                )
                hT_ps = psT.tile([P, H], f32, tag=hT_tag)
                nc.tensor.transpose(out=hT_ps[:], in_=g_sb[:], identity=ident[:])
                head_sb = hb_pool.tile([P, H], f32)
                nc.scalar.copy(out=head_sb[:], in_=hT_ps[:])

                out_sb = ob_pool.tile([P, R], f32)
                tlT = tailT_all[:, ts(i, P)]
                tl3 = tlT.rearrange("p (o k) -> p o k", o=1).to_broadcast([P, RPC, H])
                if PAIR:
                    # paired 2-bank chunks: halve the per-op overhead on the
                    # consume engines (DVE/ACT/gpsimd ops are 1024 wide)
                    tl8 = tlT.rearrange("p (o k) -> p o k", o=1).to_broadcast(
                        [P, 2 * RPC, H]
                    )
                    for jp in range(RKCH // 2):
                        tmp2 = psM.tile([P, 1024], f32, tag="tmp_ps")
                        for h2 in range(2):
                            nc.tensor.matmul(
                                out=tmp2[:, ts(h2, 512)],
                                lhsT=head_sb[:],
                                rhs=ksb[:, ts(2 * jp + h2, 512)],
                                start=True, stop=True,
                            )
                        if jp < (NDVE + 1) // 2:
                            prod = prod_pool.tile([P, 1024], f32, tag="pr0")
                            nc.vector.tensor_tensor(
                                out=prod[:].rearrange("p (r k) -> p r k", k=H),
                                in0=tmp2[:].rearrange("p (r k) -> p r k", k=H),
                                in1=tl8,
                                op=mybir.AluOpType.mult,
                            )
                            nc.vector.tensor_reduce(
                                out=out_sb[:, ts(jp, 2 * RPC)],
                                in_=prod[:].rearrange("p (r k) -> p r k", k=H),
                                axis=mybir.AxisListType.X,
                                op=mybir.AluOpType.add,
                            )
                        else:
                            cp = prod_pool.tile([P, 1024], f32, tag="cp")
                            nc.scalar.copy(out=cp[:], in_=tmp2[:])
                            pr = prod_pool.tile([P, 1024], f32, tag="pr")
                            nc.gpsimd.tensor_tensor(
                                out=pr[:].rearrange("p (r k) -> p r k", k=H),
                                in0=cp[:].rearrange("p (r k) -> p r k", k=H),
                                in1=tl8,
                                op=mybir.AluOpType.mult,
                            )
                            nc.vector.tensor_reduce(
                                out=out_sb[:, ts(jp, 2 * RPC)],
                                in_=pr[:].rearrange("p (r k) -> p r k", k=H),
                                axis=mybir.AxisListType.X,
                                op=mybir.AluOpType.add,
                            )
                    nc.sync.dma_start(out=out_ap[ts(i, P), :], in_=out_sb[:])
                    return
                for j in range(RKCH):
                    tmp_ps = psM.tile([P, 512], f32)
                    nc.tensor.matmul(
                        out=tmp_ps[:], lhsT=head_sb[:], rhs=ksb[:, ts(j, 512)],
                        start=True, stop=True,
                    )
                    if j < NDVE:
                        if STT:
                            # fused (tmp * tailT) + free-dim accum per r on DVE
                            scr = prod_pool.tile([P, 512], f32, tag="pr0")
                            for q in range(RPC):
                                r = j * RPC + q
                                nc.vector.scalar_tensor_tensor(
                                    out=scr[:, ts(q, H)],
                                    in0=tmp_ps[:, ts(q, H)],
                                    scalar=1.0,
                                    in1=tlT,
                                    op0=mybir.AluOpType.mult,
                                    op1=mybir.AluOpType.mult,
                                    accum_out=out_sb[:, r : r + 1],
                                )
                        else:
                            # DVE: tensor_tensor mul (reads tmp from PSUM) + reduce
                            prod = prod_pool.tile([P, 512], f32, tag="pr0")
                            nc.vector.tensor_tensor(
                                out=prod[:].rearrange("p (r k) -> p r k", k=H),
                                in0=tmp_ps[:].rearrange("p (r k) -> p r k", k=H),
                                in1=tl3,
                                op=mybir.AluOpType.mult,
                            )
                            nc.vector.tensor_reduce(
                                out=out_sb[:, ts(j, RPC)],
                                in_=prod[:].rearrange("p (r k) -> p r k", k=H),
                                axis=mybir.AxisListType.X,
                                op=mybir.AluOpType.add,
                            )
                    else:
                        # ACT evacuates PSUM, gpsimd multiplies, DVE reduces
                        cp = prod_pool.tile([P, 512], f32, tag="cp")
                        nc.scalar.copy(out=cp[:], in_=tmp_ps[:])
                        pr = prod_pool.tile([P, 512], f32, tag="pr")
                        nc.gpsimd.tensor_tensor(
                            out=pr[:].rearrange("p (r k) -> p r k", k=H),
                            in0=cp[:].rearrange("p (r k) -> p r k", k=H),
                            in1=tl3,
                            op=mybir.AluOpType.mult,
                        )
                        nc.vector.tensor_reduce(
                            out=out_sb[:, ts(j, RPC)],
                            in_=pr[:].rearrange("p (r k) -> p r k", k=H),
                            axis=mybir.AxisListType.X,
                            op=mybir.AluOpType.add,
                        )
                nc.sync.dma_start(out=out_ap[ts(i, P), :], in_=out_sb[:])

        if ILV:
            # tile-interleaved emission with a one-batch lag: B(b) tiles are
            # emitted right after A(b) finishes, so the consume engines chew
            # batch b while PE runs Phase A of batch b+1
            with (
                tc.tile_pool(name="xa", bufs=4) as xa_pool,
                tc.tile_pool(name="xt", bufs=3) as xt_pool,
                tc.tile_pool(name="ha", bufs=4) as ha_pool,
                tc.tile_pool(
                    name="psA",
                    bufs=(1 if PAIR else (3 if MERGEAF else 2)),
                    space="PSUM",
                ) as psA,
                tc.tile_pool(name="psF", bufs=(1 if PAIR else 2), space="PSUM") as psF,
                tc.tile_pool(name="gx", bufs=4) as gx_pool,
                tc.tile_pool(name="gb", bufs=4) as gb_pool,
                tc.tile_pool(name="hb", bufs=3) as hb_pool,
                tc.tile_pool(name="prod", bufs=4) as prod_pool,
                tc.tile_pool(name="ob", bufs=3) as ob_pool,
                tc.tile_pool(
                    name="psM",
                    bufs=(3 if PAIR else (5 if MERGEAF else 4)),
                    space="PSUM",
                ) as psM,
            ):
                TPB = S // P  # tiles per batch
                psF_eff = psA if MERGEAF else psF
                psT_eff = psA if HTA else psM
                for step in range(NTILES + TPB):
                    if step < NTILES:
                        emit_A(step, xa_pool, xt_pool, ha_pool, psA, psF_eff)
                    if step >= TPB:
                        emit_B(step - TPB, gx_pool, gb_pool, hb_pool,
                               prod_pool, ob_pool, psT_eff, psM)
        else:
            with (
                tc.tile_pool(name="xa", bufs=3) as xa_pool,
                tc.tile_pool(name="xt", bufs=2) as xt_pool,
                tc.tile_pool(name="ha", bufs=3) as ha_pool,
                tc.tile_pool(name="psA", bufs=2, space="PSUM") as psA,
                tc.tile_pool(name="psF", bufs=2, space="PSUM") as psF,
            ):
                for i in range(NTILES):
                    emit_A(i, xa_pool, xt_pool, ha_pool, psA, psF)
            # head_all DRAM writes must complete before the gathers read them
            tc.strict_bb_all_engine_barrier()
            with (
                tc.tile_pool(name="gx", bufs=3) as gx_pool,
                tc.tile_pool(name="gb", bufs=3) as gb_pool,
                tc.tile_pool(name="hb", bufs=2) as hb_pool,
                tc.tile_pool(name="prod", bufs=3) as prod_pool,
                tc.tile_pool(name="ob", bufs=3) as ob_pool,
                tc.tile_pool(name="psT", bufs=2, space="PSUM") as psT,
                tc.tile_pool(name="psM", bufs=6, space="PSUM") as psM,
            ):
                for i in range(NTILES):
                    emit_B(i, gx_pool, gb_pool, hb_pool, prod_pool, ob_pool,
                           psT, psM)

    nc.compile()
    return nc


def prep_inputs(x, head_id, root, Wh, bh, Wt, bt, kernel):
    """Host-side prep: shard over batch, precompute gather indices & root head."""
    x = np.asarray(x, dtype=np.float32)
    head_id = np.asarray(head_id)
    root = np.asarray(root, dtype=np.float32)
    Wh = np.asarray(Wh, dtype=np.float32)
    bh = np.asarray(bh, dtype=np.float32)
    Wt = np.asarray(Wt, dtype=np.float32)
    bt = np.asarray(bt, dtype=np.float32)
    kernel = np.asarray(kernel, dtype=np.float32)

    rooth = np.maximum(root @ Wh + bh, 0.0).astype(np.float32).reshape(1, H)
    shared = {
        "Wh": Wh,
        "Wt": Wt,
        "bh": bh.reshape(1, H).astype(np.float32),
        "bt": bt.reshape(1, H).astype(np.float32),
        "rooth": rooth,
        "kern": kernel,
    }
    in_maps = []
    for c in range(NCORES):
        bs = slice(c * BPC, (c + 1) * BPC)
        hid = head_id[bs].astype(np.int64)
        boff = (np.arange(BPC, dtype=np.int64) * TBL)[:, None]
        gidx = (hid + boff).reshape(TOK, 1).astype(np.int32)
        m = dict(shared)
        m["x"] = np.ascontiguousarray(x[bs].reshape(TOK, D))
        m["gidx"] = gidx
        in_maps.append(m)
    return in_maps


_NC_CACHE = {}


def _get_program(with_bias=True):
    key = ("nc", with_bias)
    if key not in _NC_CACHE:
        _NC_CACHE[key] = build_program(with_bias=with_bias)
    return _NC_CACHE[key]


def kernel(x, head_id, root, Wh, bh, Wt, bt, kernel):
    import time

    from concourse import bass_utils

    in_maps = prep_inputs(x, head_id, root, Wh, bh, Wt, bt, kernel)
    with_bias = bool(np.any(np.asarray(bh)) or np.any(np.asarray(bt)))
    nc = _get_program(with_bias=with_bias)
    res = None
    for attempt in range(6):
        try:
            res = bass_utils.run_bass_kernel_spmd(
                nc, in_maps, core_ids=list(range(NCORES))
            )
            break
        except Exception:
            # the first execution after a fresh NEFF compile (or right after
            # another session) occasionally fails at result fetch / hits a
            # transiently unrecoverable exec unit; the device recovers after
            # a short wait
            if attempt == 5:
                raise
            time.sleep(5.0 + 10.0 * attempt)
    outs = [res.results[c]["out"].reshape(BPC, S, R) for c in range(NCORES)]
    return np.concatenate(outs, axis=0)



# revision 36
# speedup vs baseline: 2.2748x; 2.2748x over previous
"""Trainium2 Bass kernel for nn_BERTSyntaxRel (biaffine syntax-relation head).

Computation (per batch b, token t):
    appended = concat([root, x[b]])                      # (S+1, D)
    gathered = appended[head_id[b, t]]                   # (D,)
    head = relu(gathered @ Wh + bh)                      # (H,)
    tail = relu(x[b, t] @ Wt + bt)                       # (H,)
    out[b, t, r] = sum_{h,k} head[h] * K[h, r, k] * tail[k]

Sharding: data-parallel over batch, 4 batches per core on 8 cores.
Weights replicated.  No collectives needed.

v2 design (vs the fp32 baseline):
  * all matmul operands in bf16 (fp32 matmul costs 4 cycles/row on TRN2's
    PE; bf16 costs 1) — tolerance is 2e-2, bf16 keeps us ~1e-3
  * x is pre-transposed on the host into [tile][dchunk, token] layout so
    the per-tile PE transposes (and their PSUM round trip) disappear
  * gather commutes with the head FF: head states for all 1025 rows per
    batch are computed once, written to a DRAM table in bf16 (256B rows)
    and gathered by token index
  * biaffine tmp = headT.T @ K lands in fp32 PSUM; the consume
    (tmp * tail, segmented-reduce over k) is split:
      - ACT evacuates most chunks PSUM->SBUF as bf16 (wide ops)
      - DVE multiplies by tail at 2x (bf16) and does one 2x fold k:128->64
      - gpsimd does the final reduce over 64
  * output written bf16, upcast on host
"""

import numpy as np

B, S, D, H, R = 32, 1024, 768, 128, 48
NCORES = 8
BPC = B // NCORES            # batches per core (4)
TOK = BPC * S                # tokens per core (4096)
P = 128                      # partition dim / token tile
NTILES = TOK // P            # 32 token tiles per core
TBL = S + 1                  # rows per batch gather table (1025)
DC = D // P                  # 6 contraction chunks of 128
RK = R * H                   # biaffine free dim (6144)
NCH = RK // 512              # 12 chunks of 512
import os as _os

ND = int(_os.environ.get("K_ND", "0"))        # chunk-pairs DVE consumes directly from
                                              # PSUM (no ACT hop)
NPL = int(_os.environ.get("K_NPL", "2"))      # chunk-pairs whose mul runs on gpsimd
                                              # (after ACT evacuation); rest: ACT->DVE 2x
XB = int(_os.environ.get("K_XB", "1"))        # xT tiles loaded per DMA
GW = int(_os.environ.get("K_GW", "8"))        # width below which the gpsimd fold chain
                                              # finishes the reduce (DVE folds to here)
PAIRW = int(_os.environ.get("K_PAIRW", "2"))  # psum tile width in banks (2 -> [P,1024])
SLABBUFS = int(_os.environ.get("K_SLABBUFS", "4"))
OBBUFS = int(_os.environ.get("K_OBBUFS", "3"))


def build_program(with_bias=True):
    """Build the Bass program (shared by all 8 cores, SPMD)."""
    from contextlib import ExitStack

    import concourse.bass as bass
    import concourse.tile as tile
    from concourse import bacc, mybir
    from concourse.masks import make_identity

    f32 = mybir.dt.float32
    bf16 = mybir.dt.bfloat16
    i32 = mybir.dt.int32
    ts = bass.ts

    nc = bacc.Bacc(
        "TRN2",
        target_bir_lowering=False,
        debug=False,
        num_devices=NCORES,
    )

    # host-prepped transposed x: per tile i, [128 dchunk-part, (c, j)] bf16
    xt_ap = nc.dram_tensor("xT", [NTILES, P, DC * P], bf16, kind="ExternalInput").ap()
    # gather indices, transposed: gidxT[p, i] = table row for token i*128+p
    gidx_ap = nc.dram_tensor("gidxT", [P, NTILES], i32, kind="ExternalInput").ap()
    wht_ap = nc.dram_tensor("wht", [P, DC * 2 * H], bf16, kind="ExternalInput").ap()
    bb_ap = nc.dram_tensor("bb", [1, 2 * H], bf16, kind="ExternalInput").ap()
    rooth_ap = nc.dram_tensor("rooth", [1, H], bf16, kind="ExternalInput").ap()
    kern_ap = nc.dram_tensor("kern", [H, RK], bf16, kind="ExternalInput").ap()
    out_ap = nc.dram_tensor("out", [TOK, R], bf16, kind="ExternalOutput").ap()

    CW = 512 * PAIRW        # psum consume-tile width (1024)
    NP = NCH // PAIRW       # number of psum consume tiles per token tile (6)
    RPW = CW // H           # r values per consume tile (8)
    TPB = S // P            # tiles per batch (8)

    with tile.TileContext(nc) as tc, ExitStack() as ctx:
        # ---- constants / weights, resident for the whole kernel ----
        const = ctx.enter_context(tc.tile_pool(name="const", bufs=1))
        ident = const.tile([P, P], bf16)
        make_identity(nc, ident[:])
        ones1 = const.tile([1, P], bf16)
        nc.gpsimd.memset(ones1[:], 1.0)
        wht = const.tile([P, DC * 2 * H], bf16)
        nc.sync.dma_start(out=wht[:], in_=wht_ap[:])
        bb_sb = const.tile([1, 2 * H], bf16)
        rt_sb = const.tile([1, H], bf16)
        nc.sync.dma_start(out=bb_sb[:], in_=bb_ap[:])
        nc.sync.dma_start(out=rt_sb[:], in_=rooth_ap[:])
        ksb = const.tile([H, RK], bf16)  # 12KB/partition
        nc.sync.dma_start(out=ksb[:], in_=kern_ap[:])

        # per-batch gather tables in DRAM: row b*TBL is the root head state
        dram = ctx.enter_context(tc.tile_pool(name="dram", bufs=1, space="DRAM"))
        head_all = dram.tile([BPC * TBL, H], bf16)
        tbl_writes = [[] for _ in range(BPC)]
        for b in range(BPC):
            w = nc.sync.dma_start(
                out=head_all[b * TBL : b * TBL + 1, :], in_=rt_sb[:1, :]
            )
            tbl_writes[b].append(w.ins)

        # ---- Phase A: FF; fills head_all (DRAM) and per-tile head|tail ----
        hts = {}

        xts = {}

        def emit_A(i, xa_pool, ha_pool, psF):
            b = i // (S // P)
            if i % XB == 0:
                xtb = xa_pool.tile([P, XB * DC * P], bf16)
                nc.sync.dma_start(
                    out=xtb[:].rearrange("p (i d) -> p i d", d=DC * P),
                    in_=xt_ap[i : i + XB].rearrange("i p d -> p i d"),
                )
                for q in range(XB):
                    xts[i + q] = xtb[:, ts(q, DC * P)]
            xt = xts.pop(i)
            ff_ps = psF.tile([P, 2 * H], f32)
            for c in range(DC):
                nc.tensor.matmul(
                    out=ff_ps[:], lhsT=xt[:, ts(c, P)], rhs=wht[:, ts(c, 2 * H)],
                    start=(c == 0), stop=(c == DC - 1 and not with_bias),
                )
            if with_bias:
                nc.tensor.matmul(
                    out=ff_ps[:], lhsT=ones1[:1, :], rhs=bb_sb[:1, :],
                    start=False, stop=True,
                )
            # one fused relu: [:, :H] = head rows (table), [:, H:] = tail
            ht = ha_pool.tile([P, 2 * H], bf16)
            nc.scalar.activation(
                out=ht[:], in_=ff_ps[:], func=mybir.ActivationFunctionType.Relu
            )
            hts[i] = ht
            row0 = b * TBL + 1 + (i % (S // P)) * P
            w = nc.sync.dma_start(out=head_all[row0 : row0 + P, :], in_=ht[:, :H])
            tbl_writes[b].append(w.ins)

        # ---- batched gather: one indirect DMA per batch, straight into an
        # SBUF tile laid out [token-within-tile(128), (tile, h)]. Offset (p,q)
        # pairs with out position (p, q*H:...): row for token q*128+p. ----
        QW = S // P  # tiles per batch (8)
        gsb_bigs = {}

        def emit_gather(b, gx_pool, gb_pool):
            gix = gx_pool.tile([P, QW], i32)
            nc.sync.dma_start(out=gix[:], in_=gidx_ap[:, b * QW : (b + 1) * QW])
            gsb = gb_pool.tile([P, QW * H], bf16)
            g = nc.gpsimd.indirect_dma_start(
                out=gsb[:].rearrange("p (q h) -> p q h", h=H),
                out_offset=None,
                in_=head_all[:],
                in_offset=bass.IndirectOffsetOnAxis(ap=gix[:, :], axis=0),
            )
            for w_ins in tbl_writes[b]:
                tile.add_dep_helper(g.ins, w_ins, sync=True, reason="head_all RAW")
            gsb_bigs[b] = gsb

        # ---- Phase B, stage 1: transpose gathered head rows ----
        headTs = {}

        def emit_B_pre(i, hb_pool, psT):
            b, q = i // QW, i % QW
            gsb = gsb_bigs[b]
            hT_ps = psT.tile([P, H], bf16)
            nc.tensor.transpose(out=hT_ps[:], in_=gsb[:, ts(q, H)], identity=ident[:])
            headT = hb_pool.tile([P, H], bf16)
            nc.vector.tensor_copy(out=headT[:], in_=hT_ps[:])
            headTs[i] = headT

        # ---- Phase B, stage 2: biaffine + consume ----
        def emit_B_main(i, slab_pool, ob_pool, psM):
            headT = headTs.pop(i)
            # biaffine: tmp[t, (r,k)] = headT.T @ K, consumed per CW-wide chunk
            slab = slab_pool.tile([P, RK], bf16, tag="slab")
            tl = hts.pop(i)[:, H:]
            tl3 = tl.rearrange("p (o k) -> p o k", o=1).to_broadcast([P, RPW, H])
            HNP = NP // 2
            # per-half consume routing: direct-DVE pairs first, then
            # ACT->gpsimd pairs, then ACT->DVE 2x pairs
            route = {}
            for q in range(ND):
                route[(q % 2) * HNP + q // 2] = "dve"
            for q in range(NPL):
                j0 = (q % 2) * HNP
                o = next(o for o in range(HNP) if (j0 + o) not in route)
                route[j0 + o] = "pool"
            out_sb = ob_pool.tile([P, R], bf16)

            def consume(j):
                tmp_ps = psM.tile([P, CW], f32)
                for hh in range(PAIRW):
                    nc.tensor.matmul(
                        out=tmp_ps[:, ts(hh, 512)],
                        lhsT=headT[:],
                        rhs=ksb[:, ts(j * PAIRW + hh, 512)],
                        start=True, stop=True,
                    )
                rt = route.get(j, "act")
                dst = slab[:, ts(j, CW)].rearrange("p (r k) -> p r k", k=H)
                if rt == "dve":
                    # DVE muls directly from PSUM (1x, fuses the evacuation)
                    nc.vector.tensor_tensor(
                        out=dst,
                        in0=tmp_ps[:].rearrange("p (r k) -> p r k", k=H),
                        in1=tl3,
                        op=mybir.AluOpType.mult,
                    )
                    return
                # ACT evacuates PSUM (casting to bf16)
                cp = slab_pool.tile([P, CW], bf16, tag="cp")
                nc.scalar.copy(out=cp[:], in_=tmp_ps[:])
                eng = nc.gpsimd if rt == "pool" else nc.vector
                eng.tensor_tensor(
                    out=dst,
                    in0=cp[:].rearrange("p (r k) -> p r k", k=H),
                    in1=tl3,
                    op=mybir.AluOpType.mult,
                )

            def reduce_half(hf):
                # reduce over k per r-half: DVE folds at 2x down to GW, then a
                # gpsimd fold chain finishes GW -> 1 into out_sb
                RH = R // 2
                with nc.allow_low_precision(reason="bf16, 2e-2 tolerance"):
                    cur3 = slab[:, ts(hf, RK // 2)].rearrange(
                        "p (r k) -> p r k", k=H
                    )
                    w = H
                    while w > 1:
                        nw = w // 2
                        eng = nc.vector if w > GW else nc.gpsimd
                        if nw > 1:
                            nxt = slab_pool.tile(
                                [P, RH * nw], bf16, tag=f"fold{nw}_{hf}"
                            )
                            o3 = nxt[:].rearrange("p (r k) -> p r k", k=nw)
                        else:
                            o3 = out_sb[:, ts(hf, RH)].rearrange(
                                "p (r one) -> p r one", one=1
                            )
                        eng.tensor_tensor(
                            out=o3,
                            in0=cur3[:, :, :nw],
                            in1=cur3[:, :, nw:w],
                            op=mybir.AluOpType.add,
                        )
                        w = nw
                        if nw > 1:
                            cur3 = nxt[:].rearrange("p (r k) -> p r k", k=w)

            for j in range(HNP):
                consume(j)
            reduce_half(0)
            for j in range(HNP, NP):
                consume(j)
            reduce_half(1)
            nc.sync.dma_start(out=out_ap[ts(i, P), :], in_=out_sb[:])

        # tile-interleaved emission with a one-batch lag: B(b) tiles are
        # emitted right after A(b) finishes, so the consume engines chew
        # batch b while PE runs Phase A of batch b+1
        with (
            tc.tile_pool(name="xa", bufs=4) as xa_pool,
            tc.tile_pool(name="ha", bufs=TPB + 4) as ha_pool,
            tc.tile_pool(name="psF", bufs=2, space="PSUM") as psF,
            tc.tile_pool(name="gx", bufs=2) as gx_pool,
            tc.tile_pool(name="gb", bufs=2) as gb_pool,
            tc.tile_pool(name="hb", bufs=3) as hb_pool,
            tc.tile_pool(name="slab", bufs=SLABBUFS) as slab_pool,
            tc.tile_pool(name="ob", bufs=OBBUFS) as ob_pool,
            tc.tile_pool(name="psT", bufs=1, space="PSUM") as psT,
            tc.tile_pool(name="psM", bufs=4 // PAIRW, space="PSUM") as psM,
        ):
            for step in range(NTILES + TPB + 1):
                if step % TPB == 0 and TPB <= step <= NTILES:
                    emit_gather(step // TPB - 1, gx_pool, gb_pool)
                if step < NTILES:
                    emit_A(step, xa_pool, ha_pool, psF)
                if TPB <= step < NTILES + TPB:
                    emit_B_pre(step - TPB, hb_pool, psT)
                if step >= TPB + 1:
                    emit_B_main(step - TPB - 1, slab_pool, ob_pool, psM)

    nc.compile()
    return nc


def _to_bf16(a):
    import ml_dtypes

    return np.asarray(a, dtype=np.float32).astype(ml_dtypes.bfloat16)


def prep_inputs(x, head_id, root, Wh, bh, Wt, bt, kernel):
    """Host-side prep: shard over batch, precompute gather indices & root head,
    cast to bf16 and pre-transpose x."""
    x = np.asarray(x, dtype=np.float32)
    head_id = np.asarray(head_id)
    root = np.asarray(root, dtype=np.float32)
    Wh = np.asarray(Wh, dtype=np.float32)
    bh = np.asarray(bh, dtype=np.float32)
    Wt = np.asarray(Wt, dtype=np.float32)
    bt = np.asarray(bt, dtype=np.float32)
    kernel = np.asarray(kernel, dtype=np.float32)

    rooth = np.maximum(root @ Wh + bh, 0.0).astype(np.float32).reshape(1, H)
    # combined FF weights: chunk c columns [c*256, c*256+256) = [Wh_c | Wt_c]
    wht = np.empty((P, DC * 2 * H), dtype=np.float32)
    for c in range(DC):
        wht[:, c * 2 * H : c * 2 * H + H] = Wh[c * P : (c + 1) * P]
        wht[:, c * 2 * H + H : (c + 1) * 2 * H] = Wt[c * P : (c + 1) * P]
    bb = np.concatenate([bh, bt]).reshape(1, 2 * H)
    shared = {
        "wht": _to_bf16(wht),
        "bb": _to_bf16(bb),
        "rooth": _to_bf16(rooth),
        "kern": _to_bf16(kernel),
    }
    in_maps = []
    for cc in range(NCORES):
        bs = slice(cc * BPC, (cc + 1) * BPC)
        hid = head_id[bs].astype(np.int64)
        boff = (np.arange(BPC, dtype=np.int64) * TBL)[:, None]
        gidx = (hid + boff).reshape(TOK).astype(np.int32)
        m = dict(shared)
        # xT[i, p, (c, j)] = x[i*128+j, c*128+p]
        xc = np.ascontiguousarray(x[bs].reshape(TOK, D))
        x4 = xc.reshape(NTILES, P, DC, P)          # [i, j, c, p]
        m["xT"] = np.ascontiguousarray(
            _to_bf16(x4.transpose(0, 3, 2, 1)))     # [i, p, c, j]
        # gidxT[p, i] = gidx[i*128 + p]
        m["gidxT"] = np.ascontiguousarray(gidx.reshape(NTILES, P).T)
        in_maps.append(m)
    return in_maps


_NC_CACHE = {}


def _get_program(with_bias=True):
    key = ("nc", with_bias)
    if key not in _NC_CACHE:
        _NC_CACHE[key] = build_program(with_bias=with_bias)
    return _NC_CACHE[key]


def kernel(x, head_id, root, Wh, bh, Wt, bt, kernel):
    import time

    from concourse import bass_utils

    in_maps = prep_inputs(x, head_id, root, Wh, bh, Wt, bt, kernel)
    with_bias = bool(np.any(np.asarray(bh)) or np.any(np.asarray(bt)))
    nc = _get_program(with_bias=with_bias)
    res = None
    for attempt in range(6):
        try:
            res = bass_utils.run_bass_kernel_spmd(
                nc, in_maps, core_ids=list(range(NCORES))
            )
            break
        except Exception:
            # the first execution after a fresh NEFF compile (or right after
            # another session) occasionally fails at result fetch; the device
            # recovers after a short wait
            if attempt == 5:
                raise
            time.sleep(5.0 + 10.0 * attempt)
    outs = [
        np.asarray(res.results[c]["out"], dtype=np.float32).reshape(BPC, S, R)
        for c in range(NCORES)
    ]
    return np.concatenate(outs, axis=0)
